# revision 1
# baseline (speedup 1.0000x reference)
"""Dilated attention (B=4,S=4096,D=768,H=12,DIL=8) on 8 TRN2 NeuronCores.

Sharding: batch x seq-half data-parallel -> core c handles batch c//2,
sequence half c%2 (2048 query tokens). The DIL-strided K/V positions
(512 per batch) are position-independent, so each core computes K/V for
its batch's 512 dilated positions locally (replicated within the
batch's core pair).

Host prep (layout only): x chunks and the dilated x are transposed to
[D, T] so the d-dim lands on SBUF partitions (PE contraction dim);
weights are passed as W.T ([in, out]). Everything SBUF-resident is
bf16 (PE full rate, half the SBUF/DMA traffic of f32r; measured
~3e-3 rel err end to end); PSUM accumulation and biases stay f32.

On-chip layouts (d-on-partition tensors are [128, 6, T] with
d = ko*128 + p):
  QT [d, t], KT [d, j]: per-head slices are [64, .] at partition base
  (h%2)*64 of d-tile h//2 -- adjacent heads row-pack the PE array.
  scores^T [j, t] per head; no max-subtraction: scores are O(1).

Softmax denominator: V is stored per head as [j, 64 v-cols | 64 ones-
cols], so the ctx matmul emits [ctx; den x64 replicated] in one PSUM
tile -- M=128 fully used, no extra PE passes.  Normalization is then a
plain per-head DVE reciprocal + multiply (no partition broadcast, no
Pool/GpSimd, no tiny DMAs), which keeps the out-projection unblocked.

Schedule: ctx(h-1) is emitted after scores(h) so ACT exp latency hides
behind PE work; the previous chunk's out-projection is injected into
the head loop; Q-proj of chunk ch+1 is emitted before ctx of the last
two heads of chunk ch so the PE crosses chunk boundaries without
waiting on the exp/normalize pipeline tail.
"""
import sys
sys.path.insert(0, "/opt/trn_rl_repo")
import numpy as np
import ml_dtypes

import concourse.bacc as bacc
import concourse.tile as tile
from concourse import mybir
from concourse.bass_utils import run_bass_kernel_spmd

B, S, D, H, DIL = 4, 4096, 768, 12, 8
HD = D // H            # 64
SD = S // DIL          # 512 dilated K/V positions
NCORE = 8
TOK = B * S // NCORE   # 2048 query tokens per core
TCH = 512              # chunk of query tokens processed at once
NCH = TOK // TCH       # 4
NKT = D // 128         # 6 contraction tiles
NJT = SD // 128        # 4 j tiles
F32 = mybir.dt.float32
BF16 = mybir.dt.bfloat16
NPBF = ml_dtypes.bfloat16
SCALE = 1.0 / float(np.sqrt(HD))
EXP = mybir.ActivationFunctionType.Exp
ADD = mybir.AluOpType.add
MULT = mybir.AluOpType.mult
DIV = mybir.AluOpType.divide

_CACHE = {}


def _head_slice(t, h, cols):
    """[64, ...] slice of a [128, NKT, T] d-on-partition tensor for head h."""
    base = (h % 2) * HD
    return t[base:base + HD, h // 2, cols]


def _build(repeat=1):
    nc = bacc.Bacc("TRN2", target_bir_lowering=False, debug=False,
                   num_devices=NCORE)

    xT_d = nc.dram_tensor("xT", [D, TOK], BF16, kind="ExternalInput")
    xdT_d = nc.dram_tensor("xdT", [D, SD], BF16, kind="ExternalInput")
    w_d = {n: nc.dram_tensor(n, [D, D], BF16, kind="ExternalInput")
           for n in ("wqT", "wkT", "wvT", "woT")}
    b_d = {n: nc.dram_tensor(n, [D], F32, kind="ExternalInput")
           for n in ("bq", "bk", "bv", "bo")}
    out_d = nc.dram_tensor("out", [TOK, D], F32, kind="ExternalOutput")

    from contextlib import ExitStack
    with tile.TileContext(nc) as tc, ExitStack() as es:
        cpool = es.enter_context(tc.tile_pool(name="const", bufs=1))
        kvpool = es.enter_context(tc.tile_pool(name="kv", bufs=2))
        xpool = es.enter_context(tc.tile_pool(name="xin", bufs=2))
        qpool = es.enter_context(tc.tile_pool(name="qt", bufs=1))
        epool = es.enter_context(tc.tile_pool(name="exps", bufs=2))
        ctpool = es.enter_context(tc.tile_pool(name="ctxt", bufs=2))
        opool = es.enter_context(tc.tile_pool(name="outs", bufs=2))
        rpool = es.enter_context(tc.tile_pool(name="rcp", bufs=2))
        mmp = es.enter_context(tc.tile_pool(name="mm", bufs=2, space="PSUM"))
        scp = es.enter_context(tc.tile_pool(name="sc", bufs=2, space="PSUM"))
        cxp = es.enter_context(tc.tile_pool(name="cx", bufs=2, space="PSUM"))

        # ---- constants: biases first (tiny; the K-proj evacuation needs bk
        # immediately), then weights in first-use order ----
        bq_sb = cpool.tile([128, NKT], F32, name="bq")
        nc.sync.dma_start(bq_sb[:], b_d["bq"].rearrange("(ko p) -> p ko", p=128))
        bk_sb = cpool.tile([128, NKT], F32, name="bk")
        nc.sync.dma_start(bk_sb[:], b_d["bk"].rearrange("(ko p) -> p ko", p=128))
        bv_sb = cpool.tile([128, D], F32, name="bv")
        nc.sync.dma_start(bv_sb[:], b_d["bv"][None, :].to_broadcast((128, D)))
        bo_sb = cpool.tile([128, D], F32, name="bo")
        nc.sync.dma_start(bo_sb[:], b_d["bo"][None, :].to_broadcast((128, D)))

        xdT_sb = cpool.tile([128, NKT, SD], BF16, name="xdT")
        _wk_r = w_d["wkT"].rearrange("(ko p) o -> p ko o", p=128)
        _xd_r = xdT_d.rearrange("(ko p) j -> p ko j", p=128)

        w_sb = {n: cpool.tile([128, NKT, D], BF16, name=n)
                for n in ("wkT", "wvT", "wqT", "woT")}
        nc.sync.dma_start(w_sb["wkT"][:, 0:2], _wk_r[:, 0:2])
        nc.sync.dma_start(xdT_sb[:, 0:2], _xd_r[:, 0:2])
        nc.sync.dma_start(w_sb["wkT"][:, 2:NKT], _wk_r[:, 2:NKT])
        nc.sync.dma_start(xdT_sb[:, 2:NKT], _xd_r[:, 2:NKT])
        nc.sync.dma_start(w_sb["wvT"][:],
                          w_d["wvT"].rearrange("(ko p) o -> p ko o", p=128))
        nc.sync.dma_start(w_sb["wqT"][:],
                          w_d["wqT"].rearrange("(ko p) o -> p ko o", p=128))
        nc.sync.dma_start(w_sb["woT"][:],
                          w_d["woT"].rearrange("(ko p) o -> p ko o", p=128))

        def emit_xdma(ch):
            xT_sb = xpool.tile([128, NKT, TCH], BF16, name="xT")
            nc.sync.dma_start(
                xT_sb[:],
                xT_d.rearrange("(ko p) t -> p ko t", p=128)
                [:, :, ch * TCH:(ch + 1) * TCH])
            return xT_sb

        def emit_qproj(xT_sb):
            qT_sb = qpool.tile([128, NKT, TCH], BF16, name="qT")
            for m in range(NKT):
                ps = mmp.tile([128, 512], F32, name="mmps")
                for kt in range(NKT):
                    nc.tensor.matmul(ps[:], w_sb["wqT"][:, kt, m * 128:(m + 1) * 128],
                                     xT_sb[:, kt, :], start=(kt == 0),
                                     stop=(kt == NKT - 1))
                nc.vector.tensor_tensor(qT_sb[:, m, :], ps[:],
                                        bq_sb[:, m, None].to_broadcast((128, TCH)), ADD)
            return qT_sb

        for _rep in range(repeat):
            # ---- K^T [d, j] ----
            kT_sb = kvpool.tile([128, NKT, SD], BF16, name="kT")
            for m in range(NKT):
                ps = mmp.tile([128, 512], F32, name="mmps")
                for kt in range(NKT):
                    nc.tensor.matmul(ps[:], w_sb["wkT"][:, kt, m * 128:(m + 1) * 128],
                                     xdT_sb[:, kt, :], start=(kt == 0),
                                     stop=(kt == NKT - 1))
                nc.vector.tensor_tensor(kT_sb[:, m, :], ps[:],
                                        bk_sb[:, m, None].to_broadcast((128, SD)), ADD)

            # ---- V [j, head, 64 v | 64 ones] (ones cols -> the ctx matmul
            # emits the softmax denominator replicated across 64 partitions,
            # with M=128 fully used and no extra PE passes) ----
            v_sb = kvpool.tile([128, NJT, H, 2 * HD], BF16, name="v")
            nc.gpsimd.memset(v_sb[:, :, :, HD:], 1.0)
            for jt in range(NJT):
                for nh0, nh1 in ((0, 8), (8, 12)):
                    ncols = (nh1 - nh0) * HD
                    ps = mmp.tile([128, 512], F32, name="mmps")
                    for kt in range(NKT):
                        nc.tensor.matmul(ps[:, :ncols],
                                         xdT_sb[:, kt, jt * 128:(jt + 1) * 128],
                                         w_sb["wvT"][:, kt, nh0 * HD:nh1 * HD],
                                         start=(kt == 0), stop=(kt == NKT - 1))
                    nc.vector.tensor_tensor(
                        v_sb[:, jt, nh0:nh1, :HD],
                        ps[:, :ncols].rearrange("p (h e) -> p h e", e=HD),
                        bv_sb[:, nh0 * HD:nh1 * HD]
                        .rearrange("p (h e) -> p h e", e=HD),
                        ADD)

            # ---- per 512-token chunk: Q^T proj, attention, out proj ----
            def emit_scores(qT_sb, h):
                exp_sb = epool.tile([128, NJT, TCH], BF16, name="expS")
                for half in range(2):
                    sp = scp.tile([128, 2, TCH], F32, name="scps")
                    for j2 in range(2):
                        jt = half * 2 + j2
                        nc.tensor.matmul(sp[:, j2, :],
                                         _head_slice(kT_sb, h,
                                                     slice(jt * 128,
                                                           (jt + 1) * 128)),
                                         _head_slice(qT_sb, h, slice(None)),
                                         start=True, stop=True)
                    nc.scalar.activation(exp_sb[:, half * 2:half * 2 + 2, :],
                                         sp[:], EXP, scale=SCALE)
                return exp_sb

            def emit_ctx(ctxT_sb, exp_sb, h):
                # cps rows 0:64 = unnormalized ctx, rows 64:128 = denominator
                # replicated x64 (from V's ones columns).  Normalize with a
                # per-head DVE reciprocal + multiply.
                cps = cxp.tile([128, TCH], F32, name="cxps")
                for jt in range(NJT):
                    nc.tensor.matmul(cps[:], v_sb[:, jt, h, :],
                                     exp_sb[:, jt, :], start=(jt == 0),
                                     stop=(jt == NJT - 1))
                rcp = rpool.tile([HD, TCH], F32, name="rcp")
                nc.vector.reciprocal(rcp[:], cps[HD:2 * HD, :])
                nc.vector.tensor_tensor(_head_slice(ctxT_sb, h, slice(None)),
                                        cps[:HD, :], rcp[:], MULT)

            def emit_out_tt(ctxT_sb, ch, tt):
                o_sb = opool.tile([128, D], F32, name="osb")
                for n0, n1 in ((0, 512), (512, 768)):
                    ps = mmp.tile([128, 512], F32, name="mmps")
                    for kt in range(NKT):
                        nc.tensor.matmul(
                            ps[:, :n1 - n0],
                            ctxT_sb[:, kt, tt * 128:(tt + 1) * 128],
                            w_sb["woT"][:, kt, n0:n1],
                            start=(kt == 0), stop=(kt == NKT - 1))
                    nc.vector.tensor_tensor(
                        o_sb[:, n0:n1], ps[:, :n1 - n0],
                        bo_sb[:, n0:n1], ADD)
                nc.sync.dma_start(out_d[ch * TCH + tt * 128:
                                        ch * TCH + (tt + 1) * 128, :], o_sb[:])

            xT_next = emit_xdma(0)
            qT_next = emit_qproj(xT_next)
            prev_ctx = None  # (ctxT_sb, ch) of previous chunk, out-proj pending
            for ch in range(NCH):
                qT_sb = qT_next
                if ch + 1 < NCH:
                    xT_next = emit_xdma(ch + 1)
                ctxT_sb = ctpool.tile([128, NKT, TCH], BF16, name="ctxT")
                pending = [0, 1, 2, 3] if prev_ctx is not None else []
                prev_exp = None
                for h in range(H):
                    exp_sb = emit_scores(qT_sb, h)
                    if h % 3 == 2 and pending:
                        emit_out_tt(prev_ctx[0], prev_ctx[1], pending.pop(0))
                    if prev_exp is not None:
                        emit_ctx(ctxT_sb, prev_exp, h - 1)
                    prev_exp = exp_sb
                # Q-proj of the next chunk before the last ctx: PE crosses the
                # chunk boundary without waiting on exp(H-1)/normalize.
                if ch + 1 < NCH:
                    qT_next = emit_qproj(xT_next)
                emit_ctx(ctxT_sb, prev_exp, H - 1)
                prev_ctx = (ctxT_sb, ch)

            for tt in range(4):
                emit_out_tt(prev_ctx[0], prev_ctx[1], tt)

    nc.compile()
    return nc


def _get_nc(repeat=1):
    if repeat not in _CACHE:
        _CACHE[repeat] = _build(repeat)
    return _CACHE[repeat]


def make_in_maps(x, Wq, bq, Wk, bk, Wv, bv, Wo, bo):
    def bf(a):
        return np.ascontiguousarray(np.asarray(a, np.float32).astype(NPBF))
    wqT = bf(np.asarray(Wq, np.float32).T)
    wkT = bf(np.asarray(Wk, np.float32).T)
    wvT = bf(np.asarray(Wv, np.float32).T)
    woT = bf(np.asarray(Wo, np.float32).T)
    x = np.asarray(x, np.float32)
    in_maps = []
    for c in range(NCORE):
        b, half = divmod(c, 2)
        xT = bf(x[b, half * TOK:(half + 1) * TOK, :].T)
        xdT = bf(x[b, ::DIL, :].T)
        in_maps.append({
            "xT": xT, "xdT": xdT,
            "wqT": wqT, "wkT": wkT, "wvT": wvT, "woT": woT,
            "bq": np.asarray(bq, np.float32), "bk": np.asarray(bk, np.float32),
            "bv": np.asarray(bv, np.float32), "bo": np.asarray(bo, np.float32),
        })
    return in_maps


def assemble(results):
    out = np.empty((B, S, D), np.float32)
    for c in range(NCORE):
        b, half = divmod(c, 2)
        out[b, half * TOK:(half + 1) * TOK, :] = results[c]["out"]
    return out


def kernel(**inputs):
    nc = _get_nc()
    in_maps = make_in_maps(**inputs)
    res = run_bass_kernel_spmd(nc, in_maps, core_ids=list(range(NCORE)))
    return assemble(res.results)

